# revision 12
# baseline (speedup 1.0000x reference)
"""Trainium2 Bass kernel for the DPPNMT seq2seq LSTM+attention model (v2).

Sharding: data-parallel over batch (64 -> 8 per core, 8 cores), params
replicated. Each core runs encoder+decoder+vocab projection+logsumexp for
its 8 batch elements; host combines per-core gold/lse partials into (64,).

v2 structural changes vs v1:
- All LSTM gate nonlinearities are Tanh (sigmoid(x) = (1+tanh(x/2))/2 with
  the 1/2 folded into packed weights; hidden states stored as H = 2h with
  the 1/2 folded into every consumer weight). The attention softmax exp is
  computed as exp(e) = 2/(1-tanh(e/2)) - 1 on DVE after a Tanh. Result:
  every activation before the final Ln lives in the single act-table set
  {Tanh, Exp, Identity, Copy} -> no per-step LoadActFuncSet (1.28us each).
- Gate order in packed weights is [g, f, i, o] so the cell is pure
  contiguous-tile ops: W = [c | tg | f i o] with one Act tanh per step.
- zx/zy bias-projections are injected into PSUM via an identity matmul
  instead of a DVE add.
- Wvocab is held resident in SBUF as fp8e4 (x64 scale, compensated by the
  exp input scale), DMA'd during the encoder; the vocab matmul+exp chunks
  are interleaved into the decoder steps so the scalar engine's exp work
  fills the decoder's dependency-chain gaps.
"""

from contextlib import ExitStack

import numpy as np
import ml_dtypes

import concourse.bass as bass
import concourse.tile as tile
from concourse import bacc, mybir
from concourse.bass_utils import run_bass_kernel_spmd
from concourse.masks import make_identity

BF16 = mybir.dt.bfloat16
F32 = mybir.dt.float32
FP8 = mybir.dt.float8e4
AF = mybir.ActivationFunctionType
ALU = mybir.AluOpType

S, T, B, E, H, V = 64, 64, 64, 256, 256, 32000
NCORES = 8
BL = B // NCORES          # local batch = 8
TD = T - 1                # decoder steps = 63
GCH = 8                   # gate chunks (4H/128)
ECH = 2
HCH = 2
NR = TD * BL              # 504 vocab rows per core
VBLK = 1536               # vocab cols per chunk (3 psum banks)
NVS = 21                  # chunks per mt pass (20 full + 1 ragged 1280)
WSCL = 64.0               # fp8 Wvocab scale
bf16 = ml_dtypes.bfloat16

HST = (S + 1) * 8
OST = (TD + 1) * 8


def build_program():
    nc = bacc.Bacc("TRN2", target_bir_lowering=False, debug=False)

    def din(name, shape, dt=BF16):
        return nc.dram_tensor(name, shape, dt, kind="ExternalInput").ap()

    xf_t = din("xf_t", [128, ECH * S * BL])
    xb_t = din("xb_t", [128, ECH * S * BL])
    wih_f = din("wih_f", [128, ECH * GCH * 128])
    wih_b = din("wih_b", [128, ECH * GCH * 128])
    whh_f = din("whh_f", [128, HCH * GCH * 128])
    whh_b = din("whh_b", [128, HCH * GCH * 128])
    benc_f = din("benc_f", [128, GCH], F32)
    benc_b = din("benc_b", [128, GCH], F32)
    yt = din("yt", [128, ECH * TD * BL])
    wihe = din("wihe", [128, ECH * GCH * 128])
    wiho = din("wiho", [128, HCH * GCH * 128])
    whhd = din("whhd", [128, HCH * GCH * 128])
    bdec = din("bdec", [128, GCH], F32)
    wcomb_l = din("wcomb_l", [128, 6 * 2 * 128])
    wh_l = din("wh_l", [128, 4 * 2 * 128])
    wc_l = din("wc_l", [128, 4 * 2 * 128])
    watt_l = din("watt_l", [128, 4 * 2 * 128])
    wvt = din("wvt", [128, HCH * V], FP8)
    wgt = din("wgt", [128, HCH * NR])
    out_lse = nc.dram_tensor("out_lse", [128, 4], F32,
                             kind="ExternalOutput").ap()
    out_gd = nc.dram_tensor("out_gd", [1, 1024], F32,
                            kind="ExternalOutput").ap()

    with tile.TileContext(nc) as tc:
        with ExitStack() as ctx:
            consts = ctx.enter_context(tc.tile_pool(name="consts", bufs=1))
            wsb = ctx.enter_context(tc.tile_pool(name="wsb", bufs=1))
            state = ctx.enter_context(tc.tile_pool(name="state", bufs=1))

            id128 = consts.tile([128, 128], BF16)
            make_identity(nc, id128[:])
            ones_bf = consts.tile([128, 1], BF16)
            nc.vector.memset(ones_bf[:], 1.0)
            ones_row = consts.tile([1, 128], BF16)
            nc.vector.memset(ones_row[:], 1.0)

            def load(ap_dram, dt=BF16):
                t = wsb.tile(list(ap_dram.shape), dt,
                             tag=ap_dram.tensor.name + "_sb")
                nc.sync.dma_start(t[:], ap_dram[:])
                return t

            xf_sb, xb_sb = load(xf_t), load(xb_t)
            wihf_sb, wihb_sb = load(wih_f), load(wih_b)
            whhf_sb, whhb_sb = load(whh_f), load(whh_b)
            bencf_sb, bencb_sb = load(benc_f, F32), load(benc_b, F32)
            yt_sb = load(yt)
            wihe_sb, wiho_sb, whhd_sb = load(wihe), load(wiho), load(whhd)
            bdec_sb = load(bdec, F32)
            wcomb_sb = load(wcomb_l)
            wh_sb, wc_sb, watt_sb = load(wh_l), load(wc_l), load(watt_l)
            wgt_sb = load(wgt)

            # resident fp8 Wvocab^T, streamed in 8 DMA pieces
            wv_sb = state.tile([128, HCH * V], FP8)
            for p in range(8):
                w0 = p * (HCH * V // 8)
                w1 = (p + 1) * (HCH * V // 8)
                nc.sync.dma_start(wv_sb[:, w0:w1], wvt[:, w0:w1])

            # persistent state
            hf_all = state.tile([128, 2 * HST], BF16)
            hb_all = state.tile([128, 2 * HST], BF16)
            for hx in (hf_all, hb_all):
                nc.vector.memset(hx[:, 0:8], 0.0)
                nc.vector.memset(hx[:, HST:HST + 8], 0.0)
            # cell tiles: [c(0:16) | tg(16:32) | f(32:48) i(48:64) o(64:80)]
            Wf = state.tile([128, 80], F32)
            Wb = state.tile([128, 80], F32)
            Wd = state.tile([128, 80], F32)
            nc.vector.memset(Wf[:, 0:16], 0.0)
            nc.vector.memset(Wb[:, 0:16], 0.0)
            outsT = state.tile([128, 2 * OST], BF16)
            nc.vector.memset(outsT[:, 0:8], 0.0)
            nc.vector.memset(outsT[:, OST:OST + 8], 0.0)
            hdec0 = state.tile([128, 16], BF16)
            zxf = state.tile([128, S * 64], BF16)
            zxb = state.tile([128, S * 64], BF16)
            zyb = state.tile([128, TD * 64], BF16)
            ehs_cs = state.tile([128, 16 * 128], BF16)
            encprojT = state.tile([128, HCH * BL * S], BF16)
            ablk = state.tile([128, 8], BF16)
            nc.vector.memset(ablk[:], 0.0)
            se_parts = state.tile([128, 4 * NVS], F32)
            nc.vector.memset(se_parts[:], 1.0)
            lse_sb = state.tile([128, 4], F32)
            gd_sb = state.tile([1, 1024], F32)
            nc.vector.memset(gd_sb[:], 0.0)
            tmp_gd = state.tile([128, 2 * NR], BF16)

            with ExitStack() as rctx:
                pep = rctx.enter_context(
                    tc.tile_pool(name="pep", bufs=2, space="PSUM"))
                pz = rctx.enter_context(
                    tc.tile_pool(name="pz", bufs=2, space="PSUM"))
                psmall = rctx.enter_context(
                    tc.tile_pool(name="psmall", bufs=1, space="PSUM"))
                work = rctx.enter_context(tc.tile_pool(name="work", bufs=2))

                # ---- zx = x @ Wih^T + b (enc, both dirs); zy likewise ----
                for (x_sb, wih_sb, b_sb, zx, nt) in (
                        (xf_sb, wihf_sb, bencf_sb, zxf, S),
                        (xb_sb, wihb_sb, bencb_sb, zxb, S),
                        (yt_sb, wihe_sb, bdec_sb, zyb, TD)):
                    zxv = zx[:].rearrange("p (t g b) -> p t g b", g=GCH, b=BL)
                    for gch in range(GCH):
                        ps = pep.tile([128, S * BL], F32, tag="pep")
                        for ech in range(ECH):
                            nc.tensor.matmul(
                                ps[:, 0:nt * BL],
                                wih_sb[:, (ech * GCH + gch) * 128:
                                       (ech * GCH + gch + 1) * 128],
                                x_sb[:, ech * nt * BL:(ech + 1) * nt * BL],
                                start=(ech == 0), stop=(ech == ECH - 1))
                        nc.scalar.activation(
                            zxv[:, 0:nt, gch, :], ps[:, 0:nt * BL],
                            AF.Identity, bias=b_sb[:, gch:gch + 1])

                # ---- encoder: two phase-shifted dir chains ----
                def cell(Wt, z, work_pfx, vec_u):
                    """z psum [128,64] (g,f,i,o) -> gates in Wt, returns
                    (tc_tile, ) after: tanh(all gates), m1/u/C2, tanh(C2/2),
                    cstore, m2, H (caller writes H)."""
                    nc.scalar.activation(Wt[:, 16:80], z[:], AF.Tanh)
                    m1 = work.tile([128, 32], F32, tag=work_pfx + "m1")
                    nc.vector.tensor_mul(m1[:], Wt[:, 32:64], Wt[:, 0:32])
                    u = work.tile([128, 32], F32, tag=work_pfx + "u")
                    vec_u.tensor_add(u[:], m1[:], Wt[:, 0:32])
                    C2 = work.tile([128, 16], F32, tag=work_pfx + "C2")
                    nc.vector.tensor_add(C2[:], u[:, 0:16], u[:, 16:32])
                    tc_ = work.tile([128, 16], F32, tag=work_pfx + "tc")
                    nc.scalar.activation(tc_[:], C2[:], AF.Tanh, scale=0.5)
                    nc.gpsimd.tensor_scalar_mul(Wt[:, 0:16], C2[:], 0.5)
                    m2 = work.tile([128, 16], F32, tag=work_pfx + "m2")
                    nc.vector.tensor_mul(m2[:], Wt[:, 64:80], tc_[:])
                    return m2, tc_

                hfv = hf_all[:].rearrange("p (c t b) -> p c t b", c=2, b=BL)
                hbv = hb_all[:].rearrange("p (c t b) -> p c t b", c=2, b=BL)
                for t in range(S):
                    for (Wt, h_all, hv, whh_sb, zx, pfx, vec_u) in (
                            (Wf, hf_all, hfv, whhf_sb, zxf, "f", nc.gpsimd),
                            (Wb, hb_all, hbv, whhb_sb, zxb, "b", nc.vector)):
                        z = pz.tile([128, 64], F32, tag="z" + pfx)
                        nc.tensor.matmul(z[:], id128[:],
                                         zx[:, t * 64:(t + 1) * 64],
                                         start=True, stop=False,
                                         skip_group_check=True)
                        for gch in range(GCH):
                            for kch in range(HCH):
                                nc.tensor.matmul(
                                    z[:, gch * 8:(gch + 1) * 8],
                                    whh_sb[:, (kch * GCH + gch) * 128:
                                           (kch * GCH + gch + 1) * 128],
                                    h_all[:, kch * HST + t * 8:
                                          kch * HST + t * 8 + 8],
                                    start=False, stop=(kch == HCH - 1),
                                    skip_group_check=True)
                        m2, tc_ = cell(Wt, z, pfx, vec_u)
                        nc.vector.tensor_add(hv[:, :, t + 1, :], m2[:], tc_[:])

                # ---- ehs_cs[64u+s, (pair,mt)] via PE transposes ----
                for pair in range(4):
                    for mt in range(4):
                        srcv = hfv if mt < 2 else hbv
                        pt = psmall.tile([128, 128], BF16, tag="ptr")
                        for u in range(2):
                            in_ap = srcv[:, mt % 2, 1:S + 1, 2 * pair + u]
                            nc.tensor.transpose(pt[u * 64:(u + 1) * 64, :],
                                                in_ap, id128[:])
                        nc.vector.tensor_copy(
                            ehs_cs[:, (pair * 4 + mt) * 128:
                                   (pair * 4 + mt + 1) * 128], pt[:])

                # ---- encproj^T = (Watt/4) @ ehs^T ----
                for mch in range(HCH):
                    ps = pep.tile([128, S * BL], F32, tag="pep")
                    for kch in range(4):
                        srch = hf_all if kch < 2 else hb_all
                        rhs = srch[:, (kch % 2) * HST + 8:
                                   (kch % 2) * HST + HST]
                        nc.tensor.matmul(
                            ps[:],
                            watt_sb[:, (kch * 2 + mch) * 128:
                                    (kch * 2 + mch + 1) * 128],
                            rhs, start=(kch == 0), stop=(kch == 3))
                    nc.scalar.activation(
                        encprojT[:, mch * BL * S:(mch + 1) * BL * S],
                        ps[:], AF.Copy)

                # ---- decoder init: dec_h/dec_c projections ----
                cfb = work.tile([128, 16], BF16, tag="cfb")
                cbb = work.tile([128, 16], BF16, tag="cbb")
                nc.vector.tensor_copy(cfb[:], Wf[:, 0:16])
                nc.vector.tensor_copy(cbb[:], Wb[:, 0:16])
                pinit = psmall.tile([128, 32], F32, tag="po")
                for (w_sb, off, hsrc, csrc) in (
                        (wh_sb, 0, (hf_all, hb_all), None),
                        (wc_sb, 16, None, (cfb, cbb))):
                    for mch in range(HCH):
                        for kch in range(4):
                            if hsrc is not None:
                                hx = hsrc[0] if kch < 2 else hsrc[1]
                                rhs = hx[:, (kch % 2) * HST + S * 8:
                                         (kch % 2) * HST + S * 8 + 8]
                            else:
                                cx = csrc[0] if kch < 2 else csrc[1]
                                rhs = cx[:, (kch % 2) * 8:(kch % 2) * 8 + 8]
                            nc.tensor.matmul(
                                pinit[:, off + mch * 8:off + (mch + 1) * 8],
                                w_sb[:, (kch * 2 + mch) * 128:
                                     (kch * 2 + mch + 1) * 128],
                                rhs, start=(kch == 0), stop=(kch == 3))
                nc.vector.tensor_copy(hdec0[:], pinit[:, 0:16])
                nc.vector.tensor_copy(Wd[:, 0:16], pinit[:, 16:32])

            # ---- decoder with interleaved vocab chunks ----
            with ExitStack() as dctx:
                pv = dctx.enter_context(
                    tc.tile_pool(name="pv", bufs=2, space="PSUM"))
                # one shared per-step psum tile (1 bank x 2 bufs):
                # cols 0:64 z | 64:96 pat | 96:112 po | 112:120 prep |
                # 120:128 peT (parts 0:64) | 128:136 pse (part 0)
                pd = dctx.enter_context(
                    tc.tile_pool(name="pd", bufs=2, space="PSUM"))
                workd = dctx.enter_context(tc.tile_pool(name="workd", bufs=2))

                epv = encprojT[:].rearrange("p (c s b) -> p c s b",
                                            c=2, b=BL)
                ovv = outsT[:].rearrange("p (c t b) -> p c t b", c=2, b=BL)
                abv = ablk[:].rearrange("p (j u) -> p j u", u=2)

                # vocab chunk machinery
                chunk_queue = []      # chunks whose mm may be emitted
                mm_done = []          # (chunk, pvt, w) awaiting exp emission

                def vocab_mm(mt, vs):
                    w = VBLK if vs < NVS - 1 else (V - (NVS - 1) * VBLK)
                    m = 128 if mt < 3 else 120
                    ntau = 16 if mt < 3 else 15
                    col0 = vs * VBLK
                    pvt = pv.tile([128, VBLK], F32, tag="pv")
                    groups = [(n0, min(512, w - n0))
                              for n0 in range(0, w, 512)]
                    for (n0, nw) in groups:
                        for ch in range(HCH):
                            lhs_ap = outsT[
                                :, ch * OST + (mt * 16 + 1) * 8:
                                ch * OST + (mt * 16 + 1 + ntau) * 8]
                            nc.tensor.matmul(
                                pvt[0:m, n0:n0 + nw],
                                lhs_ap,
                                wv_sb[:, ch * V + col0 + n0:
                                      ch * V + col0 + n0 + nw],
                                start=(ch == 0), stop=(ch == HCH - 1))
                    return (mt, vs, pvt, w, m)

                def vocab_exp(rec):
                    mt, vs, pvt, w, m = rec
                    nc.scalar.activation(
                        pvt[0:m, 0:w], pvt[0:m, 0:w], AF.Exp,
                        scale=1.0 / WSCL,
                        accum_out=se_parts[0:m, mt * NVS + vs:
                                           mt * NVS + vs + 1])

                def gates(z, h_bf, t_next, with_o):
                    """Emit inject+H-part (and O-part if with_o) for step
                    t_next into psum z."""
                    nc.tensor.matmul(z[:], id128[:],
                                     zyb[:, t_next * 64:(t_next + 1) * 64],
                                     start=True, stop=False,
                                     skip_group_check=True)
                    for gch in range(GCH):
                        for kch in range(HCH):
                            nc.tensor.matmul(
                                z[:, gch * 8:(gch + 1) * 8],
                                whhd_sb[:, (kch * GCH + gch) * 128:
                                        (kch * GCH + gch + 1) * 128],
                                h_bf[:, kch * 8:(kch + 1) * 8],
                                start=False, stop=False,
                                skip_group_check=True)
                    if with_o:
                        gates_o(z, t_next)

                def gates_o(z, t_next):
                    for gch in range(GCH):
                        for kch in range(HCH):
                            nc.tensor.matmul(
                                z[:, gch * 8:(gch + 1) * 8],
                                wiho_sb[:, (kch * GCH + gch) * 128:
                                        (kch * GCH + gch + 1) * 128],
                                outsT[:, kch * OST + t_next * 8:
                                      kch * OST + t_next * 8 + 8],
                                start=False,
                                stop=(gch == GCH - 1 and kch == HCH - 1),
                                skip_group_check=True)

                D_cur = pd.tile([128, 144], F32, tag="dec")
                gates(D_cur[:, 0:64], hdec0, 0, True)

                for t in range(TD):
                    # exp emission budget this step: chunks mm'd last step
                    exps = mm_done
                    mm_done = []
                    z_cur = D_cur[:, 0:64]
                    # -- cell --
                    nc.scalar.activation(Wd[:, 16:80], z_cur, AF.Tanh)
                    m1 = workd.tile([128, 32], F32, tag="m1")
                    nc.vector.tensor_mul(m1[:], Wd[:, 32:64], Wd[:, 0:32])
                    u = workd.tile([128, 32], F32, tag="u")
                    nc.vector.tensor_add(u[:], m1[:], Wd[:, 0:32])
                    C2 = workd.tile([128, 16], F32, tag="C2")
                    nc.vector.tensor_add(C2[:], u[:, 0:16], u[:, 16:32])
                    tc_ = workd.tile([128, 16], F32, tag="tc")
                    nc.scalar.activation(tc_[:], C2[:], AF.Tanh, scale=0.5)
                    nc.gpsimd.tensor_scalar_mul(Wd[:, 0:16], C2[:], 0.5)
                    m2 = workd.tile([128, 16], F32, tag="m2")
                    nc.vector.tensor_mul(m2[:], Wd[:, 64:80], tc_[:])
                    hdec = workd.tile([128, 16], BF16, tag="hdec")
                    nc.vector.tensor_add(hdec[:], m2[:], tc_[:])

                    # -- gates t+1: inject + H-part (O-part after tanhO) --
                    if t + 1 < TD:
                        D_next = pd.tile([128, 144], F32, tag="dec")
                        gates(D_next[:, 0:64], hdec, t + 1, False)

                    # -- Wcomb h-part (early) --
                    po = D_cur[:, 96:112]
                    for kch in (4, 5):
                        for mch in range(HCH):
                            nc.tensor.matmul(
                                po[:, mch * 8:(mch + 1) * 8],
                                wcomb_sb[:, (kch * 2 + mch) * 128:
                                         (kch * 2 + mch + 1) * 128],
                                hdec[:, (kch - 4) * 8:(kch - 3) * 8],
                                start=(kch == 4), stop=False,
                                skip_group_check=True)

                    # -- attention scores --
                    peT = D_cur[0:64, 120:128]
                    for b in range(BL):
                        for ch in range(HCH):
                            nc.tensor.matmul(
                                peT[:, b:b + 1],
                                epv[:, ch, :, b],
                                hdec[:, ch * 8 + b:ch * 8 + b + 1],
                                start=(ch == 0), stop=(ch == 1))
                    tE = workd.tile([64, 8], F32, tag="tE")
                    nc.scalar.activation(tE[:], peT, AF.Tanh, scale=0.5)
                    den = workd.tile([64, 8], F32, tag="den")
                    nc.vector.tensor_scalar(den[:], tE[:], -1.0, 1.0,
                                            ALU.mult, ALU.add)
                    rcp = workd.tile([64, 8], F32, tag="rcp")
                    nc.vector.reciprocal(rcp[:], den[:])
                    rv = rcp[:].rearrange("p (j u) -> p j u", u=2)
                    nc.vector.tensor_scalar(abv[0:64, :, 0], rv[:, :, 0],
                                            2.0, -1.0, ALU.mult, ALU.add)
                    nc.vector.tensor_scalar(abv[64:128, :, 1], rv[:, :, 1],
                                            2.0, -1.0, ALU.mult, ALU.add)

                    # first pending vocab exp sits in the tanhE->tanhO gap
                    if exps:
                        vocab_exp(exps.pop(0))

                    # -- Z and context (unnormalized) --
                    pse = D_cur[0:1, 128:136]
                    nc.tensor.matmul(pse, ones_bf[:], ablk[:],
                                     start=True, stop=True)
                    pat = D_cur[:, 64:96]
                    for pair in range(4):
                        for mt in range(4):
                            nc.tensor.matmul(
                                pat[:, mt * 8 + pair * 2:
                                    mt * 8 + pair * 2 + 2],
                                ehs_cs[:, (pair * 4 + mt) * 128:
                                       (pair * 4 + mt + 1) * 128],
                                ablk[:, 2 * pair:2 * pair + 2],
                                start=True, stop=True)
                    rz = workd.tile([1, 8], F32, tag="rz")
                    nc.vector.reciprocal(rz[:], pse)
                    rzb = workd.tile([1, 8], BF16, tag="rzb")
                    nc.vector.tensor_copy(rzb[:], rz[:])
                    prep = D_cur[:, 112:120]
                    nc.tensor.matmul(prep, ones_row[:], rzb[:],
                                     start=True, stop=True)
                    prep_s = workd.tile([128, 8], F32, tag="preps")
                    nc.vector.tensor_copy(prep_s[:], prep)
                    aTb = workd.tile([128, 32], BF16, tag="aTb")
                    pat_v = pat.rearrange("p (m b) -> p m b", b=8)
                    prep_v = prep_s[:].rearrange("p (m b) -> p m b", m=1)
                    pat_b, prep_b = bass.broadcast_tensor_aps(pat_v, prep_v)
                    aTb_v = aTb[:].rearrange("p (m b) -> p m b", b=8)
                    nc.vector.tensor_mul(aTb_v, pat_b, prep_b)

                    # -- Wcomb a-part + output tanh --
                    for kch in range(4):
                        for mch in range(HCH):
                            nc.tensor.matmul(
                                po[:, mch * 8:(mch + 1) * 8],
                                wcomb_sb[:, (kch * 2 + mch) * 128:
                                         (kch * 2 + mch + 1) * 128],
                                aTb[:, kch * 8:(kch + 1) * 8],
                                start=False,
                                stop=(kch == 3 and mch == HCH - 1),
                                skip_group_check=True)
                    nc.scalar.activation(ovv[:, :, t + 1, :],
                                         po[:, 0:16], AF.Tanh)

                    # second pending vocab exp after tanhO
                    while exps:
                        vocab_exp(exps.pop(0))

                    # -- gates t+1 O-part --
                    if t + 1 < TD:
                        gates_o(D_next[:, 0:64], t + 1)
                        D_cur = D_next

                    # -- vocab chunk mms for next step's exps --
                    if t == 15:
                        chunk_queue.extend((0, vs) for vs in range(NVS))
                    elif t == 31:
                        chunk_queue.extend((1, vs) for vs in range(NVS))
                    elif t == 47:
                        chunk_queue.extend((2, vs) for vs in range(NVS))
                    if t >= 15:
                        quota = 2 if (t - 15) % 3 == 2 else 1
                        for _ in range(quota):
                            if chunk_queue:
                                mt, vs = chunk_queue.pop(0)
                                mm_done.append(vocab_mm(mt, vs))

                # ---- vocab tail (mt=3 + leftovers) ----
                chunk_queue.extend((3, vs) for vs in range(NVS))
                for rec in mm_done:
                    vocab_exp(rec)
                mm_done = []
                for (mt, vs) in chunk_queue:
                    rec = vocab_mm(mt, vs)
                    mm_done.append(rec)
                    if len(mm_done) > 1:
                        vocab_exp(mm_done.pop(0))
                for rec in mm_done:
                    vocab_exp(rec)

                # ---- gold logits: dot(O_t, Wvocab[gold]) ----
                ovf = outsT[:].rearrange("p (c t b) -> p c t b", c=2, b=BL)
                ov = ovf[:, :, 1:, :]
                wgv = wgt_sb[:].rearrange("p (c t b) -> p c t b", c=2, b=BL)
                tgv = tmp_gd[:].rearrange("p (c t b) -> p c t b", c=2, b=BL)
                nc.vector.tensor_mul(tgv, ov, wgv)
                pgd = pv.tile([128, VBLK], F32, tag="pv")
                nc.tensor.matmul(pgd[0:1, 0:NR], ones_bf[:],
                                 tmp_gd[:, 0:NR], start=True, stop=True)
                nc.tensor.matmul(pgd[0:1, 512:512 + NR], ones_bf[:],
                                 tmp_gd[:, NR:2 * NR], start=True, stop=True)
                nc.scalar.activation(gd_sb[:, 0:NR], pgd[0:1, 0:NR], AF.Copy)
                nc.scalar.activation(gd_sb[:, 512:512 + NR],
                                     pgd[0:1, 512:512 + NR], AF.Copy)

                # ---- lse reduce + ln ----
                sev = se_parts[:].rearrange("p (mt k) -> p mt k", k=NVS)
                for mt in range(4):
                    nc.vector.tensor_reduce(
                        lse_sb[:, mt:mt + 1], sev[:, mt, :],
                        axis=mybir.AxisListType.X, op=ALU.add)
                lse2 = state.tile([128, 4], F32)
                nc.scalar.activation(lse2[:], lse_sb[:], AF.Ln)
                nc.sync.dma_start(out_lse[:], lse2[:])
                nc.sync.dma_start(out_gd[:], gd_sb[:])

    nc.compile()
    return nc


def _perm_gates(w4h, s_ifo):
    """Rows (4H, ...) in torch order i,f,g,o -> [g, f*s, i*s, o*s]."""
    wi, wf, wg, wo = np.split(np.asarray(w4h), 4, axis=0)
    return np.concatenate([wg, wf * s_ifo, wi * s_ifo, wo * s_ifo], axis=0)


def _pack_lhsT(wt, kchs, mchs):
    """wt: (K, M) = W.T -> (128, kchs*mchs*128), col=(kch*mchs+mch)*128+m."""
    tiles = [wt[k * 128:(k + 1) * 128, m * 128:(m + 1) * 128]
             for k in range(kchs) for m in range(mchs)]
    return np.ascontiguousarray(np.concatenate(tiles, axis=1)).astype(bf16)


def _pack_xT(x):
    """x: (rows, 256) -> (128, 2*rows), col = ech*rows + r."""
    a = np.ascontiguousarray(np.asarray(x).T)
    return np.ascontiguousarray(
        np.concatenate([a[:128], a[128:]], axis=1)).astype(bf16)


def _pack_bias(b):
    return np.ascontiguousarray(
        np.asarray(b).reshape(GCH, 128).T).astype(np.float32)


_NC_CACHE = {}
_RUN_KWARGS = {}      # test harness may set e.g. {"trace": True}
_LAST_RESULTS = None  # BassKernelResults of the most recent kernel() call
_LAST_INMAPS = None


def _get_program():
    if "nc" not in _NC_CACHE:
        _NC_CACHE["nc"] = build_program()
    return _NC_CACHE["nc"]


def kernel(source_padded, target_padded, src_emb, tgt_emb,
           enc_Wih_f, enc_Whh_f, enc_b_f, enc_Wih_b, enc_Whh_b, enc_b_b,
           dec_Wih, dec_Whh, dec_b, Wh, Wc, Watt, Wcomb, Wvocab):
    source_padded = np.asarray(source_padded)
    target_padded = np.asarray(target_padded)
    src_emb = np.asarray(src_emb)
    tgt_emb = np.asarray(tgt_emb)
    Wvocab = np.asarray(Wvocab)
    nc = _get_program()

    fp8 = mybir.dt.np(FP8)
    wvT = np.ascontiguousarray(Wvocab.T) * WSCL       # (256, V) * 64
    wv_packed = np.ascontiguousarray(
        np.concatenate([wvT[:128], wvT[128:]], axis=1)).astype(fp8)

    shared = {
        "wih_f": _pack_lhsT(_perm_gates(enc_Wih_f, 0.5).T, ECH, GCH),
        "wih_b": _pack_lhsT(_perm_gates(enc_Wih_b, 0.5).T, ECH, GCH),
        "whh_f": _pack_lhsT(_perm_gates(np.asarray(enc_Whh_f) * 0.5,
                                        0.5).T, HCH, GCH),
        "whh_b": _pack_lhsT(_perm_gates(np.asarray(enc_Whh_b) * 0.5,
                                        0.5).T, HCH, GCH),
        "benc_f": _pack_bias(_perm_gates(enc_b_f, 0.5)),
        "benc_b": _pack_bias(_perm_gates(enc_b_b, 0.5)),
        "wihe": _pack_lhsT(_perm_gates(np.asarray(dec_Wih)[:, :E],
                                       0.5).T, ECH, GCH),
        "wiho": _pack_lhsT(_perm_gates(np.asarray(dec_Wih)[:, E:],
                                       0.5).T, HCH, GCH),
        "whhd": _pack_lhsT(_perm_gates(np.asarray(dec_Whh) * 0.5,
                                       0.5).T, HCH, GCH),
        "bdec": _pack_bias(_perm_gates(dec_b, 0.5)),
        "wcomb_l": _pack_lhsT(np.asarray(Wcomb).T * 0.5, 6, 2),
        "wh_l": _pack_lhsT(np.asarray(Wh).T, 4, 2),
        "wc_l": _pack_lhsT(np.asarray(Wc).T, 4, 2),
        "watt_l": _pack_lhsT(np.asarray(Watt).T * 0.25, 4, 2),
        "wvt": wv_packed,
    }

    in_maps = []
    for c in range(NCORES):
        bs = slice(BL * c, BL * (c + 1))
        src = source_padded[:, bs]
        tgt = target_padded[:, bs]
        X = src_emb[src]                      # (S, 8, E)
        Y = tgt_emb[tgt[:-1]]                 # (TD, 8, E)
        wg = Wvocab[tgt[1:].reshape(-1)]      # (504, 256)
        m = dict(shared)
        m["xf_t"] = _pack_xT(X.reshape(S * BL, E))
        m["xb_t"] = _pack_xT(X[::-1].reshape(S * BL, E))
        m["yt"] = _pack_xT(Y.reshape(TD * BL, E))
        m["wgt"] = _pack_xT(wg)
        in_maps.append(m)

    r = run_bass_kernel_spmd(nc, in_maps, list(range(NCORES)),
                             **_RUN_KWARGS)
    global _LAST_RESULTS, _LAST_INMAPS
    _LAST_RESULTS = r
    _LAST_INMAPS = in_maps

    out = np.zeros(B, np.float32)
    for c in range(NCORES):
        lse = r.results[c]["out_lse"]
        gd = r.results[c]["out_gd"][0]
        lse_flat = lse.T.reshape(-1)[:NR]
        gold_logit = gd[:NR] + gd[512:512 + NR]
        p_gold = (gold_logit - lse_flat).reshape(TD, BL)
        mask = (target_padded[1:, BL * c:BL * (c + 1)] != 0)
        out[BL * c:BL * (c + 1)] = (p_gold * mask).sum(axis=0)
    return out


# revision 52
# speedup vs baseline: 1.2617x; 1.2617x over previous
"""Trainium2 Bass kernel for the DPPNMT seq2seq LSTM+attention model (v2).

Sharding: data-parallel over batch (64 -> 8 per core, 8 cores), params
replicated. Each core runs encoder+decoder+vocab projection+logsumexp for
its 8 batch elements; host combines per-core gold/lse partials into (64,).

v2 structural changes vs v1:
- All LSTM gate nonlinearities are Tanh (sigmoid(x) = (1+tanh(x/2))/2 with
  the 1/2 folded into packed weights; hidden states stored as H = 2h with
  the 1/2 folded into every consumer weight). Together with the attention
  softmax using AF.Exp directly, every activation before the final Ln
  lives in the single act-table set
  {Tanh, Exp, Identity, Copy} -> no per-step LoadActFuncSet (1.28us each).
- Gate order in packed weights is [g, f, i, o] so the cell is pure
  contiguous-tile ops: W = [c | tg | f i o] with one Act tanh per step.
- Recurrent consumers take h = m2 + tanh(c) as TWO matmul accumulation
  passes (rhs=tanh(c), then rhs=m2) so the next step / attention scores
  never wait for the h-add: encoder gate matmuls and decoder scores both
  start one DVE op earlier. zx/zy bias-projections are injected into
  PSUM via an identity matmul instead of a DVE add; the zyb projection
  phase is deferred into the early encoder to fill scalar-engine idle.
- Wvocab is held resident in SBUF as fp8e4 (x64 scale, compensated by the
  exp input scale), DMA'd during the encoder; the vocab matmul+exp chunks
  are interleaved into the decoder steps so the scalar engine's exp work
  fills the decoder's dependency-chain gaps.
"""

from contextlib import ExitStack

import numpy as np
import ml_dtypes

import concourse.bass as bass
import concourse.tile as tile
from concourse import bacc, mybir
from concourse.bass_utils import run_bass_kernel_spmd
from concourse.masks import make_identity

BF16 = mybir.dt.bfloat16
F32 = mybir.dt.float32
FP8 = mybir.dt.float8e4
AF = mybir.ActivationFunctionType
ALU = mybir.AluOpType

S, T, B, E, H, V = 64, 64, 64, 256, 256, 32000
NCORES = 8
BL = B // NCORES          # local batch = 8
TD = T - 1                # decoder steps = 63
GCH = 8                   # gate chunks (4H/128)
ECH = 2
HCH = 2
NR = TD * BL              # 504 vocab rows per core
VBLK = 512                # vocab cols per chunk (1 psum bank)
INTERLEAVE = True         # weave vocab chunks into decoder steps
NVS = 63                  # chunks per mt pass (62 full + 1 ragged 256)
WSCL = 64.0               # fp8 Wvocab scale
bf16 = ml_dtypes.bfloat16

HST = (S + 1) * 8
OST = (TD + 1) * 8


def build_program():
    nc = bacc.Bacc("TRN2", target_bir_lowering=False, debug=False)

    def din(name, shape, dt=BF16):
        return nc.dram_tensor(name, shape, dt, kind="ExternalInput").ap()

    xf_t = din("xf_t", [128, ECH * S * BL])
    xb_t = din("xb_t", [128, ECH * S * BL])
    wih_f = din("wih_f", [128, ECH * GCH * 128])
    wih_b = din("wih_b", [128, ECH * GCH * 128])
    whh_f = din("whh_f", [128, HCH * GCH * 128])
    whh_b = din("whh_b", [128, HCH * GCH * 128])
    benc_f = din("benc_f", [128, GCH], F32)
    benc_b = din("benc_b", [128, GCH], F32)
    yt = din("yt", [128, ECH * TD * BL])
    wihe = din("wihe", [128, ECH * GCH * 128])
    wiho = din("wiho", [128, HCH * GCH * 128])
    whhd = din("whhd", [128, HCH * GCH * 128])
    bdec = din("bdec", [128, GCH], F32)
    wcomb_l = din("wcomb_l", [128, 6 * 2 * 128])
    wh_l = din("wh_l", [128, 4 * 2 * 128])
    wc_l = din("wc_l", [128, 4 * 2 * 128])
    watt_l = din("watt_l", [128, 4 * 2 * 128])
    wvt = din("wvt", [128, HCH * V], FP8)
    wgt = din("wgt", [128, HCH * NR])
    out_lse = nc.dram_tensor("out_lse", [128, 4], F32,
                             kind="ExternalOutput").ap()
    out_gd = nc.dram_tensor("out_gd", [1, 1024], F32,
                            kind="ExternalOutput").ap()

    with tile.TileContext(nc) as tc:
        with ExitStack() as ctx:
            consts = ctx.enter_context(tc.tile_pool(name="consts", bufs=1))
            wsb = ctx.enter_context(tc.tile_pool(name="wsb", bufs=1))
            state = ctx.enter_context(tc.tile_pool(name="state", bufs=1))

            id128 = consts.tile([128, 128], BF16)
            make_identity(nc, id128[:])
            ones_bf = consts.tile([128, 1], BF16)
            nc.vector.memset(ones_bf[:], 1.0)
            ones_row = consts.tile([1, 128], F32)
            nc.vector.memset(ones_row[:], 1.0)

            def load(ap_dram, dt=BF16):
                t = wsb.tile(list(ap_dram.shape), dt,
                             tag=ap_dram.tensor.name + "_sb")
                nc.sync.dma_start(t[:], ap_dram[:])
                return t

            xf_sb, xb_sb = load(xf_t), load(xb_t)
            wihf_sb, wihb_sb = load(wih_f), load(wih_b)
            whhf_sb, whhb_sb = load(whh_f), load(whh_b)
            bencf_sb, bencb_sb = load(benc_f, F32), load(benc_b, F32)
            yt_sb = load(yt)
            wihe_sb, wiho_sb, whhd_sb = load(wihe), load(wiho), load(whhd)
            bdec_sb = load(bdec, F32)
            wcomb_sb = load(wcomb_l)
            wh_sb, wc_sb, watt_sb = load(wh_l), load(wc_l), load(watt_l)
            wgt_sb = load(wgt)

            # resident fp8 Wvocab^T, streamed in 8 DMA pieces
            wv_sb = state.tile([128, HCH * V], FP8)
            for p in range(8):
                w0 = p * (HCH * V // 8)
                w1 = (p + 1) * (HCH * V // 8)
                nc.sync.dma_start(wv_sb[:, w0:w1], wvt[:, w0:w1])

            # persistent state
            hf_all = state.tile([128, 2 * HST], BF16)
            hb_all = state.tile([128, 2 * HST], BF16)
            for hx in (hf_all, hb_all):
                nc.vector.memset(hx[:, 0:8], 0.0)
                nc.vector.memset(hx[:, HST:HST + 8], 0.0)
            # cell tiles: [c(0:16) | tg(16:32) | f(32:48) i(48:64) o(64:80)]
            Wf = state.tile([128, 80], F32)
            Wb = state.tile([128, 80], F32)
            Wd = state.tile([128, 80], F32)
            nc.vector.memset(Wf[:, 0:16], 0.0)
            nc.vector.memset(Wb[:, 0:16], 0.0)
            outsT = state.tile([128, 2 * OST], BF16)
            nc.vector.memset(outsT[:, 0:8], 0.0)
            nc.vector.memset(outsT[:, OST:OST + 8], 0.0)
            hdec0 = state.tile([128, 16], BF16)
            zxf = state.tile([128, S * 64], BF16)
            zxb = state.tile([128, S * 64], BF16)
            zyb = state.tile([128, TD * 64], BF16)
            ehs_cs = state.tile([128, 16 * 128], BF16)
            encprojT = state.tile([128, HCH * BL * S], BF16)
            ablk = state.tile([128, 4], BF16)
            se_parts = state.tile([128, 3 * NVS + 16], F32)
            nc.vector.memset(se_parts[:], 1.0)
            lse_sb = state.tile([128, 4], F32)
            gd_sb = state.tile([1, 1024], F32)
            nc.vector.memset(gd_sb[:], 0.0)
            tmp_gd = state.tile([128, 2 * NR], BF16)

            with ExitStack() as rctx:
                pep = rctx.enter_context(
                    tc.tile_pool(name="pep", bufs=2, space="PSUM"))
                pz = rctx.enter_context(
                    tc.tile_pool(name="pz", bufs=2, space="PSUM"))
                psmall = rctx.enter_context(
                    tc.tile_pool(name="psmall", bufs=1, space="PSUM"))
                work = rctx.enter_context(tc.tile_pool(name="work", bufs=2))

                # ---- zx = x @ Wih^T + b; zyb phase deferred past the
                # encoder so its Act work fills encoder idle ----
                def zx_phase(x_sb, wih_sb, b_sb, zx, nt):
                    zxv = zx[:].rearrange("p (t g b) -> p t g b",
                                          g=GCH, b=BL)
                    for gch in range(GCH):
                        ps = pep.tile([128, S * BL], F32, tag="pep",
                                      name="ps")
                        for ech in range(ECH):
                            nc.tensor.matmul(
                                ps[:, 0:nt * BL],
                                wih_sb[:, (ech * GCH + gch) * 128:
                                       (ech * GCH + gch + 1) * 128],
                                x_sb[:, ech * nt * BL:(ech + 1) * nt * BL],
                                start=(ech == 0), stop=(ech == ECH - 1))
                        nc.scalar.activation(
                            zxv[:, 0:nt, gch, :], ps[:, 0:nt * BL],
                            AF.Identity, bias=b_sb[:, gch:gch + 1])

                zx_phase(xf_sb, wihf_sb, bencf_sb, zxf, S)
                zx_phase(xb_sb, wihb_sb, bencb_sb, zxb, S)

                # ---- encoder: two dir chains, op-interleaved emission ----
                hfv = hf_all[:].rearrange("p (c t b) -> p c t b", c=2, b=BL)
                hbv = hb_all[:].rearrange("p (c t b) -> p c t b", c=2, b=BL)

                enc_prev = {}

                def enc_step(Wt, h_all, hv, whh_sb, zx, pfx, t):
                    z = pz.tile([128, 64], F32, tag="z" + pfx, name="z")
                    prev = enc_prev.get(pfx)
                    nc.tensor.matmul(z[:], id128[:],
                                     zx[:, t * 64:(t + 1) * 64],
                                     start=True, stop=(prev is None),
                                     skip_group_check=True)
                    if prev is not None:
                        # h_{t-1} = m2 + tc: accumulate both parts so z is
                        # ready one DVE op earlier than waiting for H
                        for src_i, hsrc in enumerate(prev):
                            for gch in range(GCH):
                                for kch in range(HCH):
                                    nc.tensor.matmul(
                                        z[:, gch * 8:(gch + 1) * 8],
                                        whh_sb[:, (kch * GCH + gch) * 128:
                                               (kch * GCH + gch + 1) * 128],
                                        hsrc[:, kch * 8:(kch + 1) * 8],
                                        start=False,
                                        stop=(src_i == 1 and kch == HCH - 1),
                                        skip_group_check=True)
                    yield
                    nc.scalar.activation(Wt[:, 16:80], z[:], AF.Tanh)
                    yield
                    m1 = work.tile([128, 32], F32, tag=pfx + "m1", name="m1")
                    nc.vector.tensor_mul(m1[:], Wt[:, 32:64], Wt[:, 0:32])
                    yield
                    u = work.tile([128, 32], F32, tag=pfx + "u", name="u")
                    nc.vector.tensor_add(u[:], m1[:], Wt[:, 0:32])
                    yield
                    C2 = work.tile([128, 16], F32, tag=pfx + "C2", name="C2")
                    nc.vector.tensor_add(C2[:], u[:, 0:16], u[:, 16:32])
                    yield
                    tc_ = work.tile([128, 16], BF16, tag=pfx + "tc",
                                    name="tc")
                    nc.scalar.activation(tc_[:], C2[:], AF.Tanh, scale=0.5)
                    yield
                    nc.gpsimd.tensor_scalar_mul(Wt[:, 0:16], C2[:], 0.5)
                    yield
                    m2 = work.tile([128, 16], BF16, tag=pfx + "m2",
                                   name="m2")
                    nc.vector.tensor_mul(m2[:], Wt[:, 64:80], tc_[:])
                    enc_prev[pfx] = (tc_, m2)
                    yield
                    nc.vector.tensor_add(hv[:, :, t + 1, :], m2[:], tc_[:])

                def drive2(*gens):
                    gens = [gn for gn in gens if gn is not None]
                    while gens:
                        keep = []
                        for gn in gens:
                            try:
                                next(gn)
                                keep.append(gn)
                            except StopIteration:
                                pass
                        gens = keep

                for t in range(S):
                    drive2(
                        enc_step(Wf, hf_all, hfv, whhf_sb, zxf, "f", t),
                        enc_step(Wb, hb_all, hbv, whhb_sb, zxb, "b", t))
                    if t == 2:
                        zx_phase(yt_sb, wihe_sb, bdec_sb, zyb, TD)

                # ---- ehs_cs[64u+s, (pair,mt)] via PE transposes ----
                for pair in range(4):
                    for mt in range(4):
                        srcv = hfv if mt < 2 else hbv
                        pt = psmall.tile([128, 128], BF16, tag="ptr")
                        for u in range(2):
                            in_ap = srcv[:, mt % 2, 1:S + 1, 2 * pair + u]
                            nc.tensor.transpose(pt[u * 64:(u + 1) * 64, :],
                                                in_ap, id128[:])
                        nc.vector.tensor_copy(
                            ehs_cs[:, (pair * 4 + mt) * 128:
                                   (pair * 4 + mt + 1) * 128], pt[:])

                # ---- encproj^T = (Watt/4) @ ehs^T ----
                for mch in range(HCH):
                    ps = pep.tile([128, S * BL], F32, tag="pep")
                    for kch in range(4):
                        srch = hf_all if kch < 2 else hb_all
                        rhs = srch[:, (kch % 2) * HST + 8:
                                   (kch % 2) * HST + HST]
                        nc.tensor.matmul(
                            ps[:],
                            watt_sb[:, (kch * 2 + mch) * 128:
                                    (kch * 2 + mch + 1) * 128],
                            rhs, start=(kch == 0), stop=(kch == 3))
                    nc.scalar.activation(
                        encprojT[:, mch * BL * S:(mch + 1) * BL * S],
                        ps[:], AF.Copy)

                # ---- decoder init: dec_h/dec_c projections ----
                cfb = work.tile([128, 16], BF16, tag="cfb")
                cbb = work.tile([128, 16], BF16, tag="cbb")
                nc.vector.tensor_copy(cfb[:], Wf[:, 0:16])
                nc.vector.tensor_copy(cbb[:], Wb[:, 0:16])
                pinit = psmall.tile([128, 32], F32, tag="po")
                for (w_sb, off, hsrc, csrc) in (
                        (wh_sb, 0, (hf_all, hb_all), None),
                        (wc_sb, 16, None, (cfb, cbb))):
                    for mch in range(HCH):
                        for kch in range(4):
                            if hsrc is not None:
                                hx = hsrc[0] if kch < 2 else hsrc[1]
                                rhs = hx[:, (kch % 2) * HST + S * 8:
                                         (kch % 2) * HST + S * 8 + 8]
                            else:
                                cx = csrc[0] if kch < 2 else csrc[1]
                                rhs = cx[:, (kch % 2) * 8:(kch % 2) * 8 + 8]
                            nc.tensor.matmul(
                                pinit[:, off + mch * 8:off + (mch + 1) * 8],
                                w_sb[:, (kch * 2 + mch) * 128:
                                     (kch * 2 + mch + 1) * 128],
                                rhs, start=(kch == 0), stop=(kch == 3))
                nc.vector.tensor_copy(hdec0[:], pinit[:, 0:16])
                nc.vector.tensor_copy(Wd[:, 0:16], pinit[:, 16:32])

            # ---- decoder with interleaved vocab chunks ----
            with ExitStack() as dctx:
                pv = dctx.enter_context(
                    tc.tile_pool(name="pv", bufs=4, space="PSUM"))
                # one shared per-step psum tile (1 bank x 2 bufs):
                # cols 0:64 z | 64:96 pat | 96:112 po | 112:120 prep |
                # 120:128 peT (parts 0:64) | 128:136 pse (part 0)
                pd = dctx.enter_context(
                    tc.tile_pool(name="pd", bufs=3, space="PSUM"))
                workd = dctx.enter_context(tc.tile_pool(name="workd", bufs=2))

                epv = encprojT[:].rearrange("p (c s b) -> p c s b",
                                            c=2, b=BL)
                ovv = outsT[:].rearrange("p (c t b) -> p c t b", c=2, b=BL)

                # vocab chunk machinery
                chunk_queue = []      # chunks whose mm may be emitted
                mm_done = []          # (chunk, pvt, w) awaiting exp emission

                def vocab_mm(mt, vs):
                    w = VBLK if vs < NVS - 1 else (V - (NVS - 1) * VBLK)
                    m = 128 if mt < 3 else 120
                    ntau = 16 if mt < 3 else 15
                    col0 = vs * VBLK
                    pvt = pv.tile([128, VBLK], F32, tag="pv")
                    groups = [(n0, min(512, w - n0))
                              for n0 in range(0, w, 512)]
                    for (n0, nw) in groups:
                        for ch in range(HCH):
                            lhs_ap = outsT[
                                :, ch * OST + (mt * 16 + 1) * 8:
                                ch * OST + (mt * 16 + 1 + ntau) * 8]
                            nc.tensor.matmul(
                                pvt[0:m, n0:n0 + nw],
                                lhs_ap,
                                wv_sb[:, ch * V + col0 + n0:
                                      ch * V + col0 + n0 + nw],
                                start=(ch == 0), stop=(ch == HCH - 1))
                    return (mt, vs, pvt, w, m)

                def vocab_exp(rec):
                    mt, vs, pvt, w, m = rec
                    nc.scalar.activation(
                        pvt[0:m, 0:w], pvt[0:m, 0:w], AF.Exp,
                        scale=1.0 / WSCL,
                        accum_out=se_parts[0:m, mt * NVS + vs:
                                           mt * NVS + vs + 1])

                zyv = zyb[:].rearrange("p (t g b) -> p t g b", g=GCH, b=BL)
                WDS = (Wd0, Wd1)
                ABS = (ablk0, ablk1)

                def gates(z, g, h_cols, t_next):
                    """inject + H-part for group g, step t_next; z [128,32]
                    cols (gch8, b4). h_cols(kch) -> [128,4] rhs."""
                    nc.tensor.matmul(z, id128[:],
                                     zyv[:, t_next, :, g * 4:g * 4 + 4],
                                     start=True, stop=False,
                                     skip_group_check=True)
                    for gch in range(GCH):
                        for kch in range(HCH):
                            nc.tensor.matmul(
                                z[:, gch * 4:(gch + 1) * 4],
                                whhd_sb[:, (kch * GCH + gch) * 128:
                                        (kch * GCH + gch + 1) * 128],
                                h_cols(kch),
                                start=False, stop=False,
                                skip_group_check=True)

                def gates_o(z, g, t_next):
                    for gch in range(GCH):
                        for kch in range(HCH):
                            nc.tensor.matmul(
                                z[:, gch * 4:(gch + 1) * 4],
                                wiho_sb[:, (kch * GCH + gch) * 128:
                                        (kch * GCH + gch + 1) * 128],
                                outsT[:, kch * OST + t_next * 8 + g * 4:
                                      kch * OST + t_next * 8 + g * 4 + 4],
                                start=False,
                                stop=(gch == GCH - 1 and kch == HCH - 1),
                                skip_group_check=True)

                # pd tile layout per group (base G = g*80):
                # z G+0:32 | pat G+32:48 | po G+48:56 | prep G+56:60 |
                # peB G+60:64 | pse G+64:68
                D_cur = pd.tile([128, 160], F32, tag="dec")
                for g in range(2):
                    gates(D_cur[:, g * 80:g * 80 + 32], g,
                          lambda kch, g=g: hdec0[:, kch * 8 + g * 4:
                                                 kch * 8 + g * 4 + 4], 0)
                    gates_o(D_cur[:, g * 80:g * 80 + 32], g, 0)

                def cell_ops(g, t, Dc, Dn):
                    """Generator: cell + gates(t+1) H-part + po-h + scores
                    + att-exps for group g, step t. Yields between ops."""
                    G = g * 80
                    Wd = WDS[g]
                    ablk = ABS[g]
                    abv = ablk[:].rearrange("p (j u) -> p j u", u=2)
                    nc.scalar.activation(Wd[:, 8:40], Dc[:, G:G + 32],
                                         AF.Tanh)
                    yield
                    m1 = workd.tile([128, 16], F32, tag=f"m1{g}")
                    nc.vector.tensor_mul(m1[:], Wd[:, 16:32], Wd[:, 0:16])
                    yield
                    u = workd.tile([128, 16], F32, tag=f"u{g}")
                    nc.vector.tensor_add(u[:], m1[:], Wd[:, 0:16])
                    yield
                    C2 = workd.tile([128, 8], F32, tag=f"C2{g}")
                    nc.vector.tensor_add(C2[:], u[:, 0:8], u[:, 8:16])
                    yield
                    tc_ = workd.tile([128, 8], F32, tag=f"tc{g}")
                    nc.scalar.activation(tc_[:], C2[:], AF.Tanh, scale=0.5)
                    yield
                    nc.gpsimd.tensor_scalar_mul(Wd[:, 0:8], C2[:], 0.5)
                    yield
                    m2 = workd.tile([128, 8], F32, tag=f"m2{g}")
                    nc.vector.tensor_mul(m2[:], Wd[:, 32:40], tc_[:])
                    yield
                    hdec = workd.tile([128, 8], BF16, tag=f"hdec{g}")
                    nc.vector.tensor_add(hdec[:], m2[:], tc_[:])
                    hds[g] = hdec
                    yield
                    if Dn is not None:
                        gates(Dn[:, G:G + 32], g,
                              lambda kch: hdec[:, kch * 4:kch * 4 + 4],
                              t + 1)
                        yield
                    po = Dc[:, G + 48:G + 56]
                    for kch in (4, 5):
                        for mch in range(HCH):
                            nc.tensor.matmul(
                                po[:, mch * 4:(mch + 1) * 4],
                                wcomb_sb[:, (kch * 2 + mch) * 128:
                                         (kch * 2 + mch + 1) * 128],
                                hdec[:, (kch - 4) * 4:(kch - 3) * 4],
                                start=(kch == 4), stop=False,
                                skip_group_check=True)
                    yield
                    peB = Dc[:, G + 60:G + 64]
                    pbv = peB.rearrange("p (j u) -> p j u", u=2)
                    for bl in range(4):
                        b = g * 4 + bl
                        u_, j_ = bl % 2, bl // 2
                        for ch in range(HCH):
                            nc.tensor.matmul(
                                peB[u_ * 64:u_ * 64 + 64,
                                    j_ * 2 + u_:j_ * 2 + u_ + 1],
                                epv[:, ch, :, b],
                                hdec[:, ch * 4 + bl:ch * 4 + bl + 1],
                                start=(ch == 0), stop=(ch == 1))
                    yield
                    nc.scalar.activation(abv[0:64, :, 0],
                                         pbv[0:64, :, 0], AF.Exp)
                    yield
                    nc.scalar.activation(abv[64:128, :, 1],
                                         pbv[64:128, :, 1], AF.Exp)

                def attn_ops(g, t, Dc, Dn):
                    """Generator: Z/context branches, aTb, Wcomb-a, tanhO,
                    gates(t+1) O-part for group g, step t."""
                    G = g * 80
                    ablk = ABS[g]
                    pse = Dc[0:1, G + 64:G + 68]
                    nc.tensor.matmul(pse, ones_bf[:], ablk[:],
                                     start=True, stop=True)
                    yield
                    pat = Dc[:, G + 32:G + 48]
                    for pl in range(2):
                        pair = g * 2 + pl
                        for mt in range(4):
                            nc.tensor.matmul(
                                pat[:, mt * 4 + pl * 2:
                                    mt * 4 + pl * 2 + 2],
                                ehs_cs[:, (pair * 4 + mt) * 128:
                                       (pair * 4 + mt + 1) * 128],
                                ablk[:, 2 * pl:2 * pl + 2],
                                start=True, stop=True)
                    yield
                    rzs = workd.tile([1, 4], F32, tag=f"rzs{g}")
                    nc.vector.reciprocal(rzs[:], pse)
                    yield
                    prep = Dc[:, G + 56:G + 60]
                    nc.tensor.matmul(prep, ones_row[:], rzs[:],
                                     start=True, stop=True)
                    yield
                    pat_s = workd.tile([128, 16], F32, tag=f"pats{g}")
                    nc.gpsimd.tensor_copy(pat_s[:], pat)
                    yield
                    aTb = workd.tile([128, 16], BF16, tag=f"aTb{g}")
                    pat_v = pat_s[:].rearrange("p (m b) -> p m b", b=4)
                    prep_v = prep.rearrange("p (m b) -> p m b", m=1)
                    pat_b, prep_b = bass.broadcast_tensor_aps(pat_v, prep_v)
                    aTb_v = aTb[:].rearrange("p (m b) -> p m b", b=4)
                    nc.vector.tensor_mul(aTb_v, pat_b, prep_b)
                    yield
                    po = Dc[:, G + 48:G + 56]
                    for kch in range(4):
                        for mch in range(HCH):
                            nc.tensor.matmul(
                                po[:, mch * 4:(mch + 1) * 4],
                                wcomb_sb[:, (kch * 2 + mch) * 128:
                                         (kch * 2 + mch + 1) * 128],
                                aTb[:, kch * 4:(kch + 1) * 4],
                                start=False,
                                stop=(kch == 3 and mch == HCH - 1),
                                skip_group_check=True)
                    yield
                    pov = po.rearrange("p (c b) -> p c b", b=4)
                    nc.scalar.activation(
                        ovv[:, :, t + 1, g * 4:g * 4 + 4], pov, AF.Tanh)
                    yield
                    if Dn is not None:
                        gates_o(Dn[:, G:G + 32], g, t + 1)

                def drive(*gens):
                    gens = [gn for gn in gens if gn is not None]
                    while gens:
                        keep = []
                        for gn in gens:
                            try:
                                next(gn)
                                keep.append(gn)
                            except StopIteration:
                                pass
                        gens = keep

                hds = [None, None]
                g1_attn = None
                for t in range(TD):
                    # vocab chunk scheduling
                    if t == 16:
                        chunk_queue.extend((0, vs) for vs in range(NVS))
                    elif t == 32:
                        chunk_queue.extend((1, vs) for vs in range(NVS))
                    elif t == 48:
                        chunk_queue.extend((2, vs) for vs in range(NVS))
                    exps = []
                    if t >= 16 and INTERLEAVE is not None and INTERLEAVE:
                        quota = 2 if (t - 15) % 8 != 7 else 3
                        for _ in range(quota):
                            if chunk_queue:
                                mt_, vs_ = chunk_queue.pop(0)
                                exps.append(vocab_mm(mt_, vs_))

                    if t + 1 < TD:
                        D_next = pd.tile([128, 160], F32, tag="dec",
                                         name="D_next")
                    else:
                        D_next = None
                    # phase A: g0 cell(t) || g1 attn(t-1)
                    drive(cell_ops(0, t, D_cur, D_next), g1_attn)
                    if exps:
                        vocab_exp(exps.pop(0))
                    # phase B: g0 attn(t) || g1 cell(t)
                    drive(attn_ops(0, t, D_cur, D_next),
                          cell_ops(1, t, D_cur, D_next))
                    while exps:
                        vocab_exp(exps.pop(0))
                    g1_attn = attn_ops(1, t, D_cur, D_next)
                    if D_next is not None:
                        D_cur = D_next
                drive(g1_attn)

                # ---- non-mt3 leftovers (small chunks) ----
                if INTERLEAVE is False:
                    chunk_queue = [(g, vs) for g in range(3)
                                   for vs in range(NVS)]
                if INTERLEAVE is None:
                    chunk_queue = []
                for (mt, vs) in chunk_queue:
                    rec = vocab_mm(mt, vs)
                    mm_done.append(rec)
                    if len(mm_done) > 1:
                        vocab_exp(mm_done.pop(0))
                for rec in mm_done:
                    vocab_exp(rec)

            # ---- mt=3 vocab tail with wide chunks, then gold + lse ----
            with ExitStack() as tctx:
                pvb = tctx.enter_context(
                    tc.tile_pool(name="pvb", bufs=2, space="PSUM"))
                VB2 = 2048
                tail_recs = []
                n2 = 16 if INTERLEAVE is not None else 0
                for vs in range(n2):
                    w = VB2 if vs < 15 else (V - 15 * VB2)
                    col0 = vs * VB2
                    pvt = pvb.tile([128, VB2], F32, tag="pvb")
                    for n0 in range(0, w, 512):
                        nw = min(512, w - n0)
                        for ch in range(HCH):
                            lhs_ap = outsT[:, ch * OST + 49 * 8:
                                           ch * OST + 64 * 8]
                            nc.tensor.matmul(
                                pvt[0:120, n0:n0 + nw],
                                lhs_ap,
                                wv_sb[:, ch * V + col0 + n0:
                                      ch * V + col0 + n0 + nw],
                                start=(ch == 0), stop=(ch == HCH - 1))
                    tail_recs.append((vs, pvt, w))
                    if len(tail_recs) > 1:
                        vs_, pvt_, w_ = tail_recs.pop(0)
                        nc.scalar.activation(
                            pvt_[0:120, 0:w_], pvt_[0:120, 0:w_], AF.Exp,
                            scale=1.0 / WSCL,
                            accum_out=se_parts[0:120, 3 * NVS + vs_:
                                               3 * NVS + vs_ + 1])
                for (vs_, pvt_, w_) in tail_recs:
                    nc.scalar.activation(
                        pvt_[0:120, 0:w_], pvt_[0:120, 0:w_], AF.Exp,
                        scale=1.0 / WSCL,
                        accum_out=se_parts[0:120, 3 * NVS + vs_:
                                           3 * NVS + vs_ + 1])

                # gold logits: dot(O_t, Wvocab[gold]) via ones-matmul
                ovf = outsT[:].rearrange("p (c t b) -> p c t b", c=2, b=BL)
                ov = ovf[:, :, 1:, :]
                wgv = wgt_sb[:].rearrange("p (c t b) -> p c t b", c=2, b=BL)
                tgv = tmp_gd[:].rearrange("p (c t b) -> p c t b", c=2, b=BL)
                nc.vector.tensor_mul(tgv, ov, wgv)
                pgd = pvb.tile([128, VB2], F32, tag="pvb", name="pgd")
                nc.tensor.matmul(pgd[0:1, 0:NR], ones_bf[:],
                                 tmp_gd[:, 0:NR], start=True, stop=True)
                nc.tensor.matmul(pgd[0:1, 512:512 + NR], ones_bf[:],
                                 tmp_gd[:, NR:2 * NR], start=True, stop=True)
                nc.scalar.activation(gd_sb[:, 0:NR], pgd[0:1, 0:NR], AF.Copy)
                nc.scalar.activation(gd_sb[:, 512:512 + NR],
                                     pgd[0:1, 512:512 + NR], AF.Copy)

                # lse reduce + ln
                for mt in range(3):
                    nc.vector.tensor_reduce(
                        lse_sb[:, mt:mt + 1],
                        se_parts[:, mt * NVS:(mt + 1) * NVS],
                        axis=mybir.AxisListType.X, op=ALU.add)
                nc.vector.tensor_reduce(
                    lse_sb[:, 3:4], se_parts[:, 3 * NVS:3 * NVS + 16],
                    axis=mybir.AxisListType.X, op=ALU.add)
                lse2 = state.tile([128, 4], F32)
                nc.scalar.activation(lse2[:], lse_sb[:], AF.Ln)
                nc.sync.dma_start(out_lse[:], lse2[:])
                nc.sync.dma_start(out_gd[:], gd_sb[:])

    nc.compile()
    return nc


def _perm_gates(w4h, s_ifo):
    """Rows (4H, ...) in torch order i,f,g,o -> [g, f*s, i*s, o*s]."""
    wi, wf, wg, wo = np.split(np.asarray(w4h), 4, axis=0)
    return np.concatenate([wg, wf * s_ifo, wi * s_ifo, wo * s_ifo], axis=0)


def _pack_lhsT(wt, kchs, mchs):
    """wt: (K, M) = W.T -> (128, kchs*mchs*128), col=(kch*mchs+mch)*128+m."""
    tiles = [wt[k * 128:(k + 1) * 128, m * 128:(m + 1) * 128]
             for k in range(kchs) for m in range(mchs)]
    return np.ascontiguousarray(np.concatenate(tiles, axis=1)).astype(bf16)


def _pack_xT(x):
    """x: (rows, 256) -> (128, 2*rows), col = ech*rows + r."""
    a = np.ascontiguousarray(np.asarray(x).T)
    return np.ascontiguousarray(
        np.concatenate([a[:128], a[128:]], axis=1)).astype(bf16)


def _pack_bias(b):
    return np.ascontiguousarray(
        np.asarray(b).reshape(GCH, 128).T).astype(np.float32)


_NC_CACHE = {}
_RUN_KWARGS = {}      # test harness may set e.g. {"trace": True}
_LAST_RESULTS = None  # BassKernelResults of the most recent kernel() call
_LAST_INMAPS = None


def _get_program():
    if "nc" not in _NC_CACHE:
        _NC_CACHE["nc"] = build_program()
    return _NC_CACHE["nc"]


def kernel(source_padded, target_padded, src_emb, tgt_emb,
           enc_Wih_f, enc_Whh_f, enc_b_f, enc_Wih_b, enc_Whh_b, enc_b_b,
           dec_Wih, dec_Whh, dec_b, Wh, Wc, Watt, Wcomb, Wvocab):
    source_padded = np.asarray(source_padded)
    target_padded = np.asarray(target_padded)
    src_emb = np.asarray(src_emb)
    tgt_emb = np.asarray(tgt_emb)
    Wvocab = np.asarray(Wvocab)
    nc = _get_program()

    fp8 = mybir.dt.np(FP8)
    wvT = np.ascontiguousarray(Wvocab.T) * WSCL       # (256, V) * 64
    wv_packed = np.ascontiguousarray(
        np.concatenate([wvT[:128], wvT[128:]], axis=1)).astype(fp8)

    shared = {
        "wih_f": _pack_lhsT(_perm_gates(enc_Wih_f, 0.5).T, ECH, GCH),
        "wih_b": _pack_lhsT(_perm_gates(enc_Wih_b, 0.5).T, ECH, GCH),
        "whh_f": _pack_lhsT(_perm_gates(np.asarray(enc_Whh_f) * 0.5,
                                        0.5).T, HCH, GCH),
        "whh_b": _pack_lhsT(_perm_gates(np.asarray(enc_Whh_b) * 0.5,
                                        0.5).T, HCH, GCH),
        "benc_f": _pack_bias(_perm_gates(enc_b_f, 0.5)),
        "benc_b": _pack_bias(_perm_gates(enc_b_b, 0.5)),
        "wihe": _pack_lhsT(_perm_gates(np.asarray(dec_Wih)[:, :E],
                                       0.5).T, ECH, GCH),
        "wiho": _pack_lhsT(_perm_gates(np.asarray(dec_Wih)[:, E:],
                                       0.5).T, HCH, GCH),
        "whhd": _pack_lhsT(_perm_gates(np.asarray(dec_Whh) * 0.5,
                                       0.5).T, HCH, GCH),
        "bdec": _pack_bias(_perm_gates(dec_b, 0.5)),
        "wcomb_l": _pack_lhsT(np.asarray(Wcomb).T * 0.5, 6, 2),
        "wh_l": _pack_lhsT(np.asarray(Wh).T, 4, 2),
        "wc_l": _pack_lhsT(np.asarray(Wc).T, 4, 2),
        "watt_l": _pack_lhsT(np.asarray(Watt).T * 0.25, 4, 2),
        "wvt": wv_packed,
    }

    in_maps = []
    for c in range(NCORES):
        bs = slice(BL * c, BL * (c + 1))
        src = source_padded[:, bs]
        tgt = target_padded[:, bs]
        X = src_emb[src]                      # (S, 8, E)
        Y = tgt_emb[tgt[:-1]]                 # (TD, 8, E)
        wg = Wvocab[tgt[1:].reshape(-1)]      # (504, 256)
        m = dict(shared)
        m["xf_t"] = _pack_xT(X.reshape(S * BL, E))
        m["xb_t"] = _pack_xT(X[::-1].reshape(S * BL, E))
        m["yt"] = _pack_xT(Y.reshape(TD * BL, E))
        m["wgt"] = _pack_xT(wg)
        in_maps.append(m)

    r = run_bass_kernel_spmd(nc, in_maps, list(range(NCORES)),
                             **_RUN_KWARGS)
    global _LAST_RESULTS, _LAST_INMAPS
    _LAST_RESULTS = r
    _LAST_INMAPS = in_maps

    out = np.zeros(B, np.float32)
    for c in range(NCORES):
        lse = r.results[c]["out_lse"]
        gd = r.results[c]["out_gd"][0]
        lse_flat = lse.T.reshape(-1)[:NR]
        gold_logit = gd[:NR] + gd[512:512 + NR]
        p_gold = (gold_logit - lse_flat).reshape(TD, BL)
        mask = (target_padded[1:, BL * c:BL * (c + 1)] != 0)
        out[BL * c:BL * (c + 1)] = (p_gold * mask).sum(axis=0)
    return out
